# revision 11
# baseline (speedup 1.0000x reference)
"""Trainium2 Bass kernel for nn_BackflowNet (gnn_message_passing) — final.

Computation per walker b (B=256, N=64, D=3):
    r_ij = x_i - x_j ; feats = [x_i, x_j, r1, r2]  (r folded into W1)
    m_ij = silu(silu(feats @ W1 + b1) @ W2 + b2)
    m_i  = sum_{j != i} m_ij
    out  = tanh(psi([x, m_i])) * bf_scale

Sharding: pure data parallel over B across 8 cores (32 walkers/core),
params replicated. HW exec ~304us in the device's fast clock mode, ~364us
in its slow mode (chip-global ~20% bimodal state observed between
processes); baseline was 460/560us. Rel err ~4e-3 (gate 2e-2).

Design (ACT-engine bound; ACT and PE both ~85% busy):
- Message MLP in fp16 (feats/W1/h/W2): 1 PE cycle/col, cheap LDWEIGHTS,
  ~1.7e-3 model error. psi layer 1 stays f32r (large m_i magnitudes),
  psi 2/3 fp16.
- Columns i-major (col = i*NJC + j): the j-sum is one DVE tensor_reduce
  over a contiguous innermost axis (strided reduce is 2x slower).
- Stacked geometry tiles walker-major (partition 3w + d). Per 16 walkers
  a combined [128, 2048] staging tile (partition 8w + featrow) is built
  with 8 strided-partition DMAs; each walker-chunk feat tile is then ONE
  contiguous 8-partition DMA at base 0, so the l1 matmul pays a single
  DMA-semaphore check (multi-level partition patterns in one DMA are
  silently broken - only 1D patterns both sides are safe).
- r2 = sum_d r_d^2 via a block-diagonal [96,32] stationary on the PE
  (cross-partition adds need PE or DMA), r1 = sqrt on ACT.
- Software-pipelined message loop: emit l1(w)+silu1(w), then l2/silu2/
  reduce of walker w-1, with feat fills prefetched 3 walkers ahead.
  silu1 runs as 2x1024-col instrs (double-buffered PSUM keeps the PE fed);
  silu2 as one 2048-col instr (its consumer is the DVE reduce, which has
  slack) - the asymmetric split measured fastest.
  Keeps the ACT queue always full and the PE in 8-matmul streaks
  (PE p-state ramps with streak length; idle gaps reset it).
- Build-chain DMAs (xi/xj geometry) are emitted before the bulk of the
  constant loads so the serial startup chain is not queued behind them.
- Self-messages zeroed via strided-column memset on m before the reduce.
- psi tail pipelined in 512-col slices.
"""

import numpy as np

B, N, D = 256, 64, 3
NCORES = 8
BW = B // NCORES        # walkers per core (32)
MSG_H = 128
HID = 128
CHUNK = 2048            # pair-cols per chunk = N i-values x NJC j-values
NJC = CHUNK // N        # j-values per chunk (32)
NCHUNK = (N * N) // CHUNK  # 2
# walker groups: 3 walkers per feat tile (bases 0/32/64)
GROUPS = [list(range(t, min(t + 3, BW))) for t in range(0, BW, 3)]


def build_program(bw=BW):
    import concourse.bass as bass
    import concourse.bacc as bacc
    import concourse.tile as tile
    import concourse.mybir as mybir

    F32 = mybir.dt.float32
    F32R = mybir.dt.float32r
    F16 = mybir.dt.float16
    AF = mybir.ActivationFunctionType
    AX = mybir.AxisListType
    ALU = mybir.AluOpType
    npart = bw * N  # particle rows for the psi stage (2048)

    nc = bacc.Bacc("TRN2", target_bir_lowering=False, debug=False)

    xf_h = nc.dram_tensor("xf", [bw, D, N], F32, kind="ExternalInput")
    w1r_h = nc.dram_tensor("w1r", [8, MSG_H], F16, kind="ExternalInput")
    sdg_h = nc.dram_tensor("sdg", [96, 32], F16, kind="ExternalInput")
    b1_h = nc.dram_tensor("b1", [MSG_H, 1], F32, kind="ExternalInput")
    w2_h = nc.dram_tensor("w2", [MSG_H, MSG_H], F16, kind="ExternalInput")
    b2_h = nc.dram_tensor("b2", [MSG_H, 1], F32, kind="ExternalInput")
    pw1m_h = nc.dram_tensor("pw1m", [MSG_H, HID], F32, kind="ExternalInput")
    pw1x_h = nc.dram_tensor("pw1x", [D, HID], F32, kind="ExternalInput")
    pb1_h = nc.dram_tensor("pb1", [HID, 1], F32, kind="ExternalInput")
    pw2_h = nc.dram_tensor("pw2", [HID, HID], F16, kind="ExternalInput")
    pb2_h = nc.dram_tensor("pb2", [HID, 1], F32, kind="ExternalInput")
    pw3_h = nc.dram_tensor("pw3", [HID, D], F16, kind="ExternalInput")
    pb3_h = nc.dram_tensor("pb3", [D, 1], F32, kind="ExternalInput")
    sc_h = nc.dram_tensor("sc", [D, 1], F32, kind="ExternalInput")
    out_h = nc.dram_tensor("out", [bw, D, N], F32, kind="ExternalOutput")

    with tile.TileContext(nc) as tc:
        with (
            tc.tile_pool(name="consts", bufs=1) as consts,
            tc.tile_pool(name="featp", bufs=1) as featp,
            tc.tile_pool(name="build", bufs=1) as build,
            tc.tile_pool(name="mid", bufs=2) as mid,
            tc.tile_pool(name="red", bufs=2) as redp,
            tc.tile_pool(name="tail", bufs=2) as tail,
            tc.tile_pool(name="ps", bufs=1, space="PSUM") as ps,
        ):
            # ---- constants ----
            w1r_t = consts.tile([8, MSG_H], F16)
            nc.sync.dma_start(out=w1r_t, in_=w1r_h.ap())
            sdg_t = consts.tile([96, 32], F16)
            nc.sync.dma_start(out=sdg_t, in_=sdg_h.ap())
            # stacked xi (walker-major: partition 3w + d), fp16, broadcast
            # over the inner j axis. Identical for both chunks.
            XIc = consts.tile([96, N], F32)
            nc.sync.dma_start(
                out=XIc,
                in_=bass.AP(xf_h, 0, [[N * D, bw], [N, D], [1, N]]),
            )
            XIch = consts.tile([96, N], F16)
            nc.vector.tensor_copy(XIch, XIc)
            XIb = consts.tile([96, CHUNK], F16)
            nc.vector.tensor_copy(
                XIb.rearrange("p (i j) -> p i j", j=NJC),
                XIch.unsqueeze(2).broadcast_to([96, N, NJC]),
            )

            def build_chunk(c):
                """Stacked fp16 xj/r1/r2 for chunk c (partition 3w + d)."""
                XJc = build.tile([96, NJC], F32, tag="xjc", bufs=2)
                nc.sync.dma_start(
                    out=XJc,
                    in_=bass.AP(xf_h, NJC * c,
                                [[N * D, bw], [N, D], [1, NJC]]),
                )
                XJch = build.tile([96, NJC], F16, tag="xjch", bufs=2)
                nc.vector.tensor_copy(XJch, XJc)
                XJ = build.tile([96, CHUNK], F16, tag="xj", bufs=2)
                nc.vector.tensor_copy(
                    XJ.rearrange("p (i j) -> p i j", j=NJC),
                    XJch.unsqueeze(1).broadcast_to([96, N, NJC]),
                )
                R = build.tile([96, CHUNK], F16, tag="r", bufs=1)
                nc.vector.tensor_sub(R, XIb, XJ)
                nc.vector.tensor_mul(R, R, R)
                r1c = build.tile([32, CHUNK], F16, tag="r1", bufs=2)
                r2c = build.tile([32, CHUNK], F16, tag="r2", bufs=2)
                pr = ps.tile([MSG_H, CHUNK], F32, tag="o2", bufs=1,
                             name="pr")
                for k in range(4):
                    nc.tensor.matmul(
                        pr[0:32, 512 * k:512 * (k + 1)], sdg_t,
                        R[:, 512 * k:512 * (k + 1)],
                        start=True, stop=True)
                nc.scalar.activation(r1c, pr[0:32, :],
                                     AF.Sqrt, bias=eps_t, scale=1.0)
                nc.vector.tensor_copy(r2c, pr[0:32, :])
                return XJ, r1c, r2c

            eps_t = consts.tile([32, 1], F32)
            nc.vector.memset(eps_t, 1e-12)

            chunks = [build_chunk(0), build_chunk(1)]

            w2_t = consts.tile([MSG_H, MSG_H], F16)
            nc.sync.dma_start(out=w2_t, in_=w2_h.ap())

            def load_f16(h, shape, tag):
                t = consts.tile(shape, F16, tag=tag, name=tag)
                nc.sync.dma_start(out=t, in_=h.ap())
                return t

            def load_f32r(h, shape, tag):
                f = consts.tile(shape, F32, tag="stage", bufs=2, name="stage")
                nc.sync.dma_start(out=f, in_=h.ap())
                t = consts.tile(shape, F32R, tag=tag, name=tag)
                nc.vector.tensor_copy(t, f)
                return t

            pw1m_t = load_f32r(pw1m_h, [MSG_H, HID], "pw1m")
            pw1x_t = load_f32r(pw1x_h, [D, HID], "pw1x")
            pw2_t = load_f16(pw2_h, [HID, HID], "pw2")
            pw3_t = load_f16(pw3_h, [HID, D], "pw3")
            b1_t = consts.tile([MSG_H, 1], F32)
            nc.sync.dma_start(out=b1_t, in_=b1_h.ap())
            b2_t = consts.tile([MSG_H, 1], F32)
            nc.sync.dma_start(out=b2_t, in_=b2_h.ap())
            pb1_t = consts.tile([HID, 1], F32)
            nc.sync.dma_start(out=pb1_t, in_=pb1_h.ap())
            pb2_t = consts.tile([HID, 1], F32)
            nc.sync.dma_start(out=pb2_t, in_=pb2_h.ap())
            pb3_t = consts.tile([D, 1], F32)
            nc.sync.dma_start(out=pb3_t, in_=pb3_h.ap())
            sc_t = consts.tile([D, 1], F32)
            nc.sync.dma_start(out=sc_t, in_=sc_h.ap())
            # xT: [3, bw*64] for psi, col = w*64 + i
            xT_f = consts.tile([D, npart], F32, tag="xtf")
            nc.sync.dma_start(
                out=xT_f,
                in_=bass.AP(xf_h, 0, [[N, D], [N * D, bw], [1, N]]),
            )
            xT_t = consts.tile([D, npart], F32R)
            nc.vector.tensor_copy(xT_t, xT_f)
            # m_i accumulator, col = w*64 + i
            stash_t = consts.tile([MSG_H, npart], F32)


            def fill_cmb(xjt, r1c, r2c, half):
                """Assemble 16 walkers x 8 feature rows (partition 8k + r)
                from the stacked build tiles; 8 strided-partition DMAs."""
                w0 = 16 * half
                cmb = featp.tile([128, CHUNK], F16, tag="cmb", bufs=2,
                                 name="cmb")
                for d in range(D):
                    nc.sync.dma_start(
                        out=cmb[d:128:8, :],
                        in_=XIb[3 * w0 + d:3 * (w0 + 16):3, :])
                    nc.sync.dma_start(
                        out=cmb[3 + d:128:8, :],
                        in_=xjt[3 * w0 + d:3 * (w0 + 16):3, :])
                nc.sync.dma_start(out=cmb[6:128:8, :],
                                  in_=r2c[w0:w0 + 16, :])
                nc.sync.dma_start(out=cmb[7:128:8, :],
                                  in_=r1c[w0:w0 + 16, :])
                return cmb

            def fill_feat(cmb, k):
                """One contiguous DMA: walker k's 8 rows -> base-0 tile."""
                fg = featp.tile([8, CHUNK], F16, tag="feat", bufs=6,
                                name="feat")
                nc.sync.dma_start(out=fg, in_=cmb[8 * k:8 * k + 8, :])
                return fg

            def stage_l1(fg):
                """l1 matmuls + silu -> h for one walker-chunk."""
                h = mid.tile([MSG_H, CHUNK], F16, tag="h", name="h")
                for half in range(2):
                    base = 1024 * half
                    o1 = ps.tile([MSG_H, 1024], F32, tag="o1", bufs=2,
                                 name="o1")
                    for k in range(2):
                        s = base + 512 * k
                        nc.tensor.matmul(
                            o1[:, 512 * k:512 * (k + 1)],
                            w1r_t,
                            fg[:, s:s + 512],
                            start=True, stop=True)
                    nc.scalar.activation(
                        h[:, base:base + 1024], o1,
                        AF.Silu, bias=b1_t, scale=1.0)
                return h

            def stage_l2(c, w, h):
                """l2 matmuls + silu + mask + j-sum for one walker-chunk."""
                m = mid.tile([MSG_H, CHUNK], F16, tag="m", name="m", bufs=3)
                o2 = ps.tile([MSG_H, CHUNK], F32, tag="o2", bufs=1, name="o2")
                for k in range(4):
                    s = 512 * k
                    nc.tensor.matmul(o2[:, s:s + 512], w2_t, h[:, s:s + 512],
                                     start=True, stop=True)
                nc.scalar.activation(m, o2, AF.Silu, bias=b2_t, scale=1.0)
                # zero self-messages: i-major diag cols 33*ii + 1024*c
                d0 = NJC * NJC * c
                nc.vector.memset(
                    m[:, d0:min(d0 + 33 * NJC, CHUNK):N // 2 + 1], 0.0)
                # j-sum: reduce contiguous innermost of [128, i=64, j=32]
                if c == 0:
                    nc.vector.tensor_reduce(
                        stash_t[:, N * w:N * (w + 1)],
                        m.rearrange("p (i j) -> p i j", j=NJC),
                        AX.X, ALU.add)
                else:
                    t = redp.tile([MSG_H, N], F32, tag="t", name="t")
                    nc.vector.tensor_reduce(
                        t, m.rearrange("p (i j) -> p i j", j=NJC),
                        AX.X, ALU.add)
                    nc.vector.tensor_add(
                        stash_t[:, N * w:N * (w + 1)],
                        stash_t[:, N * w:N * (w + 1)], t)

            # Software-pipelined message loop (walker granularity):
            # l1(w)+silu1(w), then l2/silu2/reduce of walker w-1. Walker
            # feat tiles are single-DMA fills from the CMB staging tiles,
            # prefetched 3 walkers ahead.
            cmbs = {}
            for c in (0, 1):
                for half in (0, 1):
                    xjt, r1c, r2c = chunks[c]
                    cmbs[(c, half)] = None  # filled lazily below
            work = [(c, w) for c in (0, 1) for w in range(bw)]

            def get_cmb(c, w):
                key = (c, w // 16)
                if cmbs[key] is None:
                    xjt, r1c, r2c = chunks[c]
                    cmbs[key] = fill_cmb(xjt, r1c, r2c, w // 16)
                return cmbs[key]

            PRE = 3
            fgs = {}
            for pre in range(PRE):
                ci, wi = work[pre]
                fgs[pre] = fill_feat(get_cmb(ci, wi), wi % 16)
            prev = None
            for idx, (c, w) in enumerate(work):
                if idx + PRE < len(work):
                    ci, wi = work[idx + PRE]
                    fgs[idx + PRE] = fill_feat(get_cmb(ci, wi), wi % 16)
                fg = fgs.pop(idx)
                h = stage_l1(fg)
                if prev is not None:
                    stage_l2(*prev)
                prev = (c, w, h)
            stage_l2(*prev)
            # ---- psi MLP, pipelined per 512-col slice (fp16) ----
            for k0 in range(0, npart, 512):
                sl = slice(k0, k0 + 512)
                sr = tail.tile([MSG_H, 512], F32R, tag="sr", name="sr")
                nc.vector.tensor_copy(sr, stash_t[:, sl])
                pp_t = ps.tile([HID, 1024], F32, tag="o1", bufs=2,
                               name="pp_t")
                pp = pp_t[:, 0:512]
                nc.tensor.matmul(pp, pw1m_t, sr, start=True, stop=False)
                nc.tensor.matmul(pp, pw1x_t, xT_t[:, sl],
                                 start=False, stop=True)
                u1 = tail.tile([HID, 512], F16, tag="u1", name="u1")
                nc.scalar.activation(u1, pp, AF.Silu, bias=pb1_t, scale=1.0)
                pp2_t = ps.tile([HID, CHUNK], F32, tag="o2", bufs=1,
                                name="pp2_t")
                pp2 = pp2_t[:, 0:512]
                nc.tensor.matmul(pp2, pw2_t, u1, start=True, stop=True)
                u2 = tail.tile([HID, 512], F16, tag="u2", name="u2")
                nc.scalar.activation(u2, pp2, AF.Silu, bias=pb2_t, scale=1.0)
                pd_t = ps.tile([HID, 1024], F32, tag="o1", bufs=2,
                               name="pd_t")
                pd = pd_t[0:D, 0:512]
                nc.tensor.matmul(pd, pw3_t, u2, start=True, stop=True)
                dxs = tail.tile([D, 512], F32, tag="dxs", name="dxs")
                nc.scalar.activation(dxs, pd, AF.Tanh, bias=pb3_t, scale=1.0)
                nc.vector.tensor_scalar_mul(dxs, dxs, sc_t)
                nc.sync.dma_start(
                    out=bass.AP(out_h, (k0 // N) * N * D,
                                [[N, D], [N * D, 512 // N], [1, N]]),
                    in_=dxs.rearrange("p (w i) -> p w i", i=N),
                )
    nc.compile()
    return nc


def host_inputs(x, phi_w1, phi_b1, phi_w2, phi_b2,
                psi_w1, psi_b1, psi_w2, psi_b2, psi_w3, psi_b3, bf_scale,
                bw=BW, ncores=NCORES):
    """Per-core in_maps from the full problem inputs."""
    x = np.asarray(x, np.float32)
    w1 = np.asarray(phi_w1, np.float64)
    w1p = np.concatenate([
        w1[0:3] + w1[6:9],      # xi rows (r folded in)
        w1[3:6] - w1[6:9],      # xj rows
        w1[10:11],              # r2
        w1[9:10],               # r1
    ], axis=0).astype(np.float16)
    w1r = np.ascontiguousarray(w1p)
    sdg = np.zeros((96, 32), np.float16)
    for d in range(D):
        for w in range(32):
            sdg[3 * w + d, w] = 1.0
    sc = np.maximum(np.float32(bf_scale), 0.0)
    const = {
        "w1r": w1r,
        "sdg": sdg,
        "b1": np.asarray(phi_b1, np.float32).reshape(MSG_H, 1),
        "w2": np.asarray(phi_w2, np.float16),
        "b2": np.asarray(phi_b2, np.float32).reshape(MSG_H, 1),
        "pw1x": np.ascontiguousarray(np.asarray(psi_w1, np.float32)[0:3]),
        "pw1m": np.ascontiguousarray(np.asarray(psi_w1, np.float32)[3:]),
        "pb1": np.asarray(psi_b1, np.float32).reshape(HID, 1),
        "pw2": np.asarray(psi_w2, np.float16),
        "pb2": np.asarray(psi_b2, np.float32).reshape(HID, 1),
        "pw3": np.asarray(psi_w3, np.float16),
        "pb3": np.asarray(psi_b3, np.float32).reshape(D, 1),
        "sc": np.full((D, 1), sc, np.float32),
    }
    in_maps = []
    for core in range(ncores):
        xs = np.ascontiguousarray(
            x[core * bw:(core + 1) * bw].transpose(0, 2, 1))
        in_maps.append({"xf": xs, **const})
    return in_maps


_cached_nc = None
LAST_EXEC_NS = None


def kernel(x, spin, phi_w1, phi_b1, phi_w2, phi_b2,
           psi_w1, psi_b1, psi_w2, psi_b2, psi_w3, psi_b3, bf_scale):
    global _cached_nc
    from concourse.bass_utils import run_bass_kernel_spmd

    if _cached_nc is None:
        _cached_nc = build_program()
    in_maps = host_inputs(x, phi_w1, phi_b1, phi_w2, phi_b2,
                          psi_w1, psi_b1, psi_w2, psi_b2, psi_w3, psi_b3,
                          bf_scale)
    res = run_bass_kernel_spmd(_cached_nc, in_maps, core_ids=list(range(NCORES)))
    global LAST_EXEC_NS
    if res.exec_time_ns is not None:
        LAST_EXEC_NS = res.exec_time_ns
    out = np.concatenate(
        [r["out"].transpose(0, 2, 1) for r in res.results], axis=0)
    return out.astype(np.float32)
